# revision 2
# baseline (speedup 1.0000x reference)
"""Binary-tree gated-expert MoE kernel for 8 Trainium2 NeuronCores.

Reference computation (B=4096, D=2048, 4 levels, 1/2/4/8 experts):
    h = x
    for level l: h = relu(h @ Wl[eid_l] + bl[eid_l])
where eid_l is the l-bit prefix of the 3-bit leaf id built from
path_mask[:, 0:3].

Strategy: expert-parallel over the 8 leaves with host-side dispatch.
Sorting samples by leaf id makes every level's expert groups contiguous
(level-l ids are prefixes of the leaf id), so core c processes leaf
group c and needs exactly 4 weight matrices: W0[0], W1[c>>2], W2[c>>1],
W3[c].  Groups are Binomial(B, 1/8) ~ 512+-21 rows; each is padded to a
common per-core batch Bc, processed as column chunks sized to the PSUM
bank / matmul moving limit (512), e.g. (512, 32) for Bc=544.

Everything on-device runs in float16: fp16 matmuls stream at 1 col/cy
with ~10 cy/instr overhead (vs ~43 cy for fp32r), and weight DMA
halves.  fp32 accumulation in PSUM keeps the error ~5e-4.  Activations
stay transposed [D, Bc] in SBUF across all levels (output partition dim
= output features, so no transposes anywhere).  Weights stream
HBM->SBUF per 512-column group, double buffered.
"""

import math

import numpy as np

from concourse import bacc, mybir, tile
from concourse.bass_utils import run_bass_kernel_spmd

D = 2048
KT = D // 128          # 16 contraction k-tiles
JT = D // 128          # 16 output-feature blocks
JG = 4                 # j-groups of 4 blocks (512 features) per W DMA
N_CORES = 8
N_LEVELS = 4
F32 = mybir.dt.float32
F16 = mybir.dt.float16

_cache: dict = {}


def _build(chunks: tuple):
    """Build + compile the per-core Bass program for batch Bc = sum(chunks)."""
    key = chunks
    if key in _cache:
        return _cache[key]
    Bc = sum(chunks)
    csl = []
    off = 0
    for ch in chunks:
        csl.append(slice(off, off + ch))
        off += ch

    nc = bacc.Bacc("TRN2", target_bir_lowering=False, debug=False,
                   num_devices=N_CORES)

    # Weights arrive host-linearized as [JG, 128, KT*512]:
    # element (jg, p, kt, jc) = W[kt*128 + p, jg*512 + jc], so each DMA
    # reads long contiguous runs per partition.
    xT = nc.dram_tensor("xT", [D, Bc], F16, kind="ExternalInput")
    Ws = [nc.dram_tensor(f"W{l}", [JG, 128, KT * 512], F16,
                         kind="ExternalInput")
          for l in range(N_LEVELS)]
    bias = nc.dram_tensor("bias", [N_LEVELS, D], F32, kind="ExternalInput")
    out = nc.dram_tensor("out", [D, Bc], F16, kind="ExternalOutput")

    xTv = xT.rearrange("(kt p) b -> p kt b", p=128)
    outv = out.rearrange("(jt p) b -> p jt b", p=128)
    bv = bias.rearrange("l (jt p) -> p l jt", p=128)
    NQ = 4                      # W DMA split: 4 quarters of 4 k-tiles
    KQ = KT // NQ               # k-tiles per quarter
    QW = KQ * 512               # W free-dim elements per quarter
    PACE_WIN = 3                # max in-flight paced DMAs on the SP ring

    with tile.TileContext(nc) as tc:
        with (
            tc.tile_pool(name="acts", bufs=1) as acts,
            tc.tile_pool(name="w", bufs=3) as wpool,
            tc.tile_pool(name="ps", bufs=8, space="PSUM") as ps,
            tc.tile_pool(name="misc", bufs=1) as misc,
        ):
            actA = acts.tile([128, KT, Bc], F16, tag="A")
            actB = acts.tile([128, KT, Bc], F16, tag="B")
            btile = misc.tile([128, N_LEVELS, JT], F32)
            nc.scalar.dma_start(btile[:], bv)

            # Warm the PE HAM clock gate during the DMA lead-in: ~6us of
            # throwaway matmuls on a zeroed tile so the first real matmul
            # runs at 2.4GHz instead of 1.2GHz.
            warm = misc.tile([128, 512], F16)
            nc.gpsimd.memset(warm[:], 0.0)
            wacc = ps.tile([128, 512], F32, tag="ps", name="wacc")
            for _ in range(30):
                nc.tensor.matmul(wacc[:], warm[:, :128], warm[:],
                                 start=True, stop=True)

            # All bulk input DMAs go on the SP ring, chained so at most
            # PACE_WIN are in flight.  The HW SDMA engines round-robin
            # packets across every queued transfer, so an unbounded
            # backlog makes every transfer finish near the end; a short
            # chain keeps completion order = consumption order with the
            # stream still running at full HBM rate.
            paced = []

            def paced_dma(dst_ap, src_ap):
                h = nc.sync.dma_start(dst_ap, src_ap)
                # tighter window while the first matmul's inputs stream
                win = 2 if len(paced) < 4 else PACE_WIN
                if len(paced) >= win:
                    tile.add_dep_helper(h.ins, paced[-win].ins,
                                        reason="dma pacing chain")
                paced.append(h)
                return h

            # x pieces, emitted interleaved with the first weight
            # quarters in consumption-priority order.  The first piece
            # (k-quarter 0, chunk 0) is all the first matmul needs.
            pend_x = []
            if len(chunks) > 1:
                pend_x.append((slice(0, KQ), slice(chunks[0], Bc)))
            for q in range(1, NQ):
                pend_x.append((slice(q * KQ, (q + 1) * KQ), slice(0, Bc)))
            paced_dma(actA[:, 0:KQ, csl[0]], xTv[:, 0:KQ, csl[0]])

            for l in range(N_LEVELS):
                src = actA if l % 2 == 0 else actB
                dst = actB if l % 2 == 0 else actA
                for jg in range(JG):
                    wt = wpool.tile([128, KT, 4 * 128], F16, tag="w")
                    wflat = wt.rearrange("p kt j -> p (kt j)")
                    accs = [ps.tile([128, ch], F32, tag="ps", name="acc")
                            for ch in chunks for _ in range(4)]
                    for q in range(NQ):
                        paced_dma(
                            wflat[:, q * QW:(q + 1) * QW],
                            Ws[l][jg][:, q * QW:(q + 1) * QW])
                        if l == 0 and jg == 0 and pend_x:
                            ks, cs_x = pend_x.pop(0)
                            paced_dma(actA[:, ks, cs_x], xTv[:, ks, cs_x])
                        for c, cs in enumerate(csl):
                            for jj in range(4):
                                acc = accs[c * 4 + jj]
                                for kt in range(q * KQ, (q + 1) * KQ):
                                    nc.tensor.matmul(
                                        acc[:],
                                        wt[:, kt, jj * 128:(jj + 1) * 128],
                                        src[:, kt, cs],
                                        start=(kt == 0),
                                        stop=(kt == KT - 1),
                                    )
                    for c, cs in enumerate(csl):
                        for jj in range(4):
                            jt = jg * 4 + jj
                            acc = accs[c * 4 + jj]
                            nc.scalar.activation(
                                dst[:, jt, cs], acc[:],
                                mybir.ActivationFunctionType.Relu,
                                bias=btile[:, l, jt:jt + 1],
                            )
                    if l == N_LEVELS - 1:
                        # final level: dst == actA; ship this jg's four
                        # feature blocks via SWDGE (GpSimd) so the store
                        # never head-of-line-blocks the paced W chain.
                        # Last jg goes out per chunk so the tail DMA is
                        # small.
                        if jg < JG - 1:
                            nc.gpsimd.dma_start(
                                outv[:, jg * 4:(jg + 1) * 4, :],
                                dst[:, jg * 4:(jg + 1) * 4, :])
                        else:
                            for c, cs in enumerate(csl):
                                for jj in range(4):
                                    jt = jg * 4 + jj
                                    last = (c == len(chunks) - 1 and jj == 3)
                                    eng = nc.scalar if last else nc.gpsimd
                                    eng.dma_start(
                                        outv[:, jt, cs],
                                        dst[:, jt, cs])

    nc.compile()
    _cache[key] = nc
    return nc


def _linearize_w(W: np.ndarray) -> np.ndarray:
    """[D, D] -> [JG, 128, KT*512] with (jg, p, kt, jc) = W[kt*128+p, jg*512+jc]."""
    return np.ascontiguousarray(
        W.reshape(KT, 128, JG, 512).transpose(2, 1, 0, 3).reshape(
            JG, 128, KT * 512))


def _plan(path_mask: np.ndarray):
    pm = np.asarray(path_mask)
    e3 = (pm[:, 0] * 4 + pm[:, 1] * 2 + pm[:, 2]).astype(np.int64)
    counts = np.bincount(e3, minlength=N_CORES)
    maxg = int(max(counts.max(), 1))
    # Column chunks: matmul moving limit / PSUM bank = 512 fp32.  Use
    # full 512-col chunks (lowest per-instruction overhead) plus one
    # small tail chunk.
    full, rem = divmod(maxg, 512)
    chunks = (512,) * full
    if rem:
        chunks = chunks + (max(16, (rem + 7) // 8 * 8),)
    Bc = sum(chunks)
    return e3, maxg, Bc, chunks


def kernel(x, path_mask, W0, b0, W1, b1, W2, b2, W3, b3, _trace=False):
    x = np.asarray(x, dtype=np.float32)
    Wls = [np.asarray(W, dtype=np.float32) for W in (W0, W1, W2, W3)]
    bls = [np.asarray(b, dtype=np.float32) for b in (b0, b1, b2, b3)]
    B = x.shape[0]

    e3, maxg, Bc, chunks = _plan(path_mask)
    nseg = math.ceil(maxg / Bc)
    nc = _build(chunks)

    xT16 = np.ascontiguousarray(x.T.astype(np.float16))
    core_rows = [np.nonzero(e3 == c)[0] for c in range(N_CORES)]
    wb_maps = []
    for c in range(N_CORES):
        eids = (0, c >> 2, c >> 1, c)
        wb_maps.append({
            **{f"W{l}": _linearize_w(Wls[l][eids[l]].astype(np.float16))
               for l in range(N_LEVELS)},
            "bias": np.ascontiguousarray(
                np.stack([bls[l][eids[l]] for l in range(N_LEVELS)])),
        })

    out_full = np.zeros((B, D), dtype=np.float32)
    last_res = None
    for s in range(nseg):
        in_maps = []
        for c in range(N_CORES):
            rows = core_rows[c][s * Bc:(s + 1) * Bc]
            xTc = np.zeros((D, Bc), dtype=np.float16)
            xTc[:, :len(rows)] = xT16[:, rows]
            in_maps.append({"xT": xTc, **wb_maps[c]})
        res = run_bass_kernel_spmd(nc, in_maps, list(range(N_CORES)),
                                   trace=_trace)
        last_res = res
        for c in range(N_CORES):
            rows = core_rows[c][s * Bc:(s + 1) * Bc]
            out_full[rows] = res.results[c]["out"][:, :len(rows)].T.astype(
                np.float32)
    if _trace:
        return out_full, last_res
    return out_full


# revision 3
# speedup vs baseline: 1.0006x; 1.0006x over previous
"""Binary-tree gated-expert MoE kernel for 8 Trainium2 NeuronCores.

Reference computation (B=4096, D=2048, 4 levels, 1/2/4/8 experts):
    h = x
    for level l: h = relu(h @ Wl[eid_l] + bl[eid_l])
where eid_l is the l-bit prefix of the 3-bit leaf id built from
path_mask[:, 0:3].

Strategy: expert-parallel over the 8 leaves with host-side dispatch.
Core c processes leaf group c, needing weights W0[0], W1[c>>2],
W2[c>>1], W3[c].  Leaf groups are Binomial(B, 1/8) ~ 512+-21 rows, but
the matmul moving-dim / PSUM-bank limit is 512 columns, so each core
runs one 512-column primary chunk plus a small tail chunk of T columns
(Bc = 512 + T).  A leaf's overflow beyond 512 rows goes to its own tail
or to its SIBLING core's tail: siblings share W1/W2, so only level 3
needs a second weight matrix (W3sec input) for the tail chunk.  This
keeps every core at exactly <= Bc real columns with near-perfect load
balance and full-width (lowest-overhead) matmuls.

Everything on-device runs in float16: fp16 matmuls stream at 1 col/cy
with ~10 cy/instr overhead (vs ~43 cy for fp32r), and weight DMA
halves.  fp32 accumulation in PSUM keeps the error ~5e-4.  Activations
stay transposed [D, Bc] in SBUF across all levels (output partition dim
= output features, so no transposes anywhere).  Weights stream
HBM->SBUF per 512-column group, double buffered.

Falls back to a generic multi-chunk leaf-per-core build for extreme
routing skew (leaf > 512 + 2T rows or sibling pair > 1024 + 2T).
"""

import math

import numpy as np

from concourse import bacc, mybir, tile
from concourse.bass_utils import run_bass_kernel_spmd

D = 2048
KT = D // 128          # 16 contraction k-tiles
JT = D // 128          # 16 output-feature blocks
JG = 4                 # j-groups of 4 blocks (512 features) per W DMA
N_CORES = 8
N_LEVELS = 4
F32 = mybir.dt.float32
F16 = mybir.dt.float16

_cache: dict = {}


def _build(chunks: tuple, tail_w3sec: bool):
    """Per-core Bass program, batch Bc = sum(chunks).

    chunks: column chunk sizes (each <= 512).  If tail_w3sec, the last
    chunk uses a separate W3sec weight stream at level 3.
    """
    key = (chunks, tail_w3sec)
    if key in _cache:
        return _cache[key]
    Bc = sum(chunks)
    csl = []
    off = 0
    for ch in chunks:
        csl.append(slice(off, off + ch))
        off += ch

    nc = bacc.Bacc("TRN2", target_bir_lowering=False, debug=False,
                   num_devices=N_CORES)

    # Weights arrive host-linearized as [JG, 128, KT*512]:
    # element (jg, p, kt, jc) = W[kt*128 + p, jg*512 + jc], so each DMA
    # reads long contiguous runs per partition.
    xT = nc.dram_tensor("xT", [D, Bc], F16, kind="ExternalInput")
    Ws = [nc.dram_tensor(f"W{l}", [JG, 128, KT * 512], F16,
                         kind="ExternalInput")
          for l in range(N_LEVELS)]
    if tail_w3sec:
        W3s = nc.dram_tensor("W3sec", [JG, 128, KT * 512], F16,
                             kind="ExternalInput")
    bias = nc.dram_tensor("bias", [N_LEVELS, D], F32, kind="ExternalInput")
    out = nc.dram_tensor("out", [D, Bc], F16, kind="ExternalOutput")

    xTv = xT.rearrange("(kt p) b -> p kt b", p=128)
    outv = out.rearrange("(jt p) b -> p jt b", p=128)
    bv = bias.rearrange("l (jt p) -> p l jt", p=128)
    NQ = 4                      # W DMA split: 4 quarters of 4 k-tiles
    KQ = KT // NQ               # k-tiles per quarter
    QW = KQ * 512               # W free-dim elements per quarter
    PACE_WIN = 3                # max in-flight paced DMAs on the SP ring

    with tile.TileContext(nc) as tc:
        with (
            tc.tile_pool(name="acts", bufs=1) as acts,
            tc.tile_pool(name="w", bufs=3) as wpool,
            tc.tile_pool(name="ps", bufs=8, space="PSUM") as ps,
            tc.tile_pool(name="misc", bufs=1) as misc,
        ):
            actA = acts.tile([128, KT, Bc], F16, tag="A")
            actB = acts.tile([128, KT, Bc], F16, tag="B")
            btile = misc.tile([128, N_LEVELS, JT], F32)
            nc.scalar.dma_start(btile[:], bv)

            # Warm the PE HAM clock gate during the DMA lead-in with
            # throwaway matmuls on a zeroed tile so the first real
            # matmul runs at full clock instead of 1.2GHz.
            warm = misc.tile([128, 256], F16)
            nc.gpsimd.memset(warm[:], 0.0)
            wacc = ps.tile([128, 256], F32, tag="ps", name="wacc")
            for _ in range(24):
                nc.tensor.matmul(wacc[:], warm[:, :128], warm[:],
                                 start=True, stop=True)

            # All bulk input DMAs go on the SP ring, chained so at most
            # PACE_WIN are in flight.  The HW SDMA engines round-robin
            # packets across every queued transfer, so an unbounded
            # backlog makes every transfer finish near the end; a short
            # chain keeps completion order = consumption order with the
            # stream still running at full HBM rate.
            paced = []

            def paced_dma(dst_ap, src_ap):
                h = nc.sync.dma_start(dst_ap, src_ap)
                # tighter window while the first matmul's inputs stream
                win = 2 if len(paced) < 4 else PACE_WIN
                if len(paced) >= win:
                    tile.add_dep_helper(h.ins, paced[-win].ins,
                                        reason="dma pacing chain")
                paced.append(h)
                return h

            # x streams in k-quarter pieces; the first piece is all the
            # first matmuls need, the rest interleave with jg0 weight
            # quarters in consumption-priority order.
            pend_x = [(slice(q * KQ, (q + 1) * KQ), slice(0, Bc))
                      for q in range(1, NQ)]
            paced_dma(actA[:, 0:KQ, :], xTv[:, 0:KQ, :])

            for l in range(N_LEVELS):
                src = actA if l % 2 == 0 else actB
                dst = actB if l % 2 == 0 else actA
                use_sec = tail_w3sec and l == N_LEVELS - 1
                for jg in range(JG):
                    wt = wpool.tile([128, KT, 4 * 128], F16, tag="w")
                    wflat = wt.rearrange("p kt j -> p (kt j)")
                    if use_sec:
                        wt2 = wpool.tile([128, KT, 4 * 128], F16, tag="w2")
                        wflat2 = wt2.rearrange("p kt j -> p (kt j)")
                    accs = [ps.tile([128, ch], F32, tag="ps", name="acc")
                            for ch in chunks for _ in range(4)]
                    for q in range(NQ):
                        paced_dma(
                            wflat[:, q * QW:(q + 1) * QW],
                            Ws[l][jg][:, q * QW:(q + 1) * QW])
                        if use_sec:
                            paced_dma(
                                wflat2[:, q * QW:(q + 1) * QW],
                                W3s[jg][:, q * QW:(q + 1) * QW])
                        if l == 0 and jg == 0 and pend_x:
                            ks, cs_x = pend_x.pop(0)
                            paced_dma(actA[:, ks, cs_x], xTv[:, ks, cs_x])
                        for c, cs in enumerate(csl):
                            wsrc = wt2 if (use_sec and c == len(csl) - 1) \
                                else wt
                            for jj in range(4):
                                acc = accs[c * 4 + jj]
                                for kt in range(q * KQ, (q + 1) * KQ):
                                    nc.tensor.matmul(
                                        acc[:],
                                        wsrc[:, kt, jj * 128:(jj + 1) * 128],
                                        src[:, kt, cs],
                                        start=(kt == 0),
                                        stop=(kt == KT - 1),
                                    )
                    last_jg = l == N_LEVELS - 1 and jg == JG - 1
                    for c, cs in enumerate(csl):
                        for jj in range(4):
                            jt = jg * 4 + jj
                            acc = accs[c * 4 + jj]
                            if last_jg and jj % 2 == 1:
                                # split the drain-phase relus across the
                                # Act and DVE engines to halve the
                                # serial tail after the last matmul.
                                nc.vector.tensor_scalar(
                                    dst[:, jt, cs], acc[:],
                                    btile[:, l, jt:jt + 1], 0.0,
                                    mybir.AluOpType.add,
                                    mybir.AluOpType.max)
                            else:
                                nc.scalar.activation(
                                    dst[:, jt, cs], acc[:],
                                    mybir.ActivationFunctionType.Relu,
                                    bias=btile[:, l, jt:jt + 1],
                                )
                    if l == N_LEVELS - 1:
                        # final level: dst == actA; ship this jg's four
                        # feature blocks via SWDGE (GpSimd) so the store
                        # never head-of-line-blocks the paced W chain.
                        # Last jg goes out per (chunk, jt) so the tail
                        # DMAs are small and start early.
                        if jg < JG - 1:
                            nc.gpsimd.dma_start(
                                outv[:, jg * 4:(jg + 1) * 4, :],
                                dst[:, jg * 4:(jg + 1) * 4, :])
                        else:
                            for c, cs in enumerate(csl):
                                for jj in range(4):
                                    jt = jg * 4 + jj
                                    last = (c == len(chunks) - 1 and jj == 3)
                                    eng = nc.scalar if last else nc.gpsimd
                                    eng.dma_start(
                                        outv[:, jt, cs],
                                        dst[:, jt, cs])

    nc.compile()
    _cache[key] = nc
    return nc


def _linearize_w(W: np.ndarray) -> np.ndarray:
    """[D, D] -> [JG, 128, KT*512] with (jg, p, kt, jc) = W[kt*128+p, jg*512+jc]."""
    return np.ascontiguousarray(
        W.reshape(KT, 128, JG, 512).transpose(2, 1, 0, 3).reshape(
            JG, 128, KT * 512))


def _plan(path_mask: np.ndarray):
    """Choose the per-core column layout.

    Returns (mode, T, placements) where placements[c] =
    (prim_rows, tail_rows, w3sec_eid); prim_rows go to columns
    [0:len), tail_rows to columns [512:512+len).  mode 'legacy' means
    fall back to leaf-per-core multi-chunk (placements is (e3, maxg)).
    """
    pm = np.asarray(path_mask)
    e3 = (pm[:, 0] * 4 + pm[:, 1] * 2 + pm[:, 2]).astype(np.int64)
    counts = np.bincount(e3, minlength=N_CORES)
    leaf_rows = [np.nonzero(e3 == c)[0] for c in range(N_CORES)]
    maxg = int(max(counts.max(), 1))

    for T in (16, 24, 32, 48, 64):
        ok = True
        placements = []
        for k in range(N_CORES // 2):
            a, b = 2 * k, 2 * k + 1
            na, nb = int(counts[a]), int(counts[b])
            oa, ob = max(0, na - 512), max(0, nb - 512)
            if na > 512 + 2 * T or nb > 512 + 2 * T or \
                    na + nb > 1024 + 2 * T or (oa > T and ob > 0) or \
                    (ob > T and oa > 0):
                ok = False
                break
            ra, rb = leaf_rows[a], leaf_rows[b]
            # own overflow first into own tail, remainder to sibling
            ta = min(oa, T)
            tb = min(ob, T)
            spill_a = oa - ta          # a rows going to b's tail
            spill_b = ob - tb
            pa = (ra[:512], np.concatenate([ra[512:512 + ta], rb[nb - spill_b:]]),
                  a if spill_b == 0 else b)
            pb = (rb[:512], np.concatenate([rb[512:512 + tb], ra[na - spill_a:]]),
                  b if spill_a == 0 else a)
            if spill_a and spill_b:
                ok = False
                break
            placements.extend([pa, pb])
        if ok:
            return "tail", T, placements, e3
    return "legacy", 0, (e3, maxg), e3


def kernel(x, path_mask, W0, b0, W1, b1, W2, b2, W3, b3, _trace=False):
    x = np.asarray(x, dtype=np.float32)
    Wls = [np.asarray(W, dtype=np.float32) for W in (W0, W1, W2, W3)]
    bls = [np.asarray(b, dtype=np.float32) for b in (b0, b1, b2, b3)]
    B = x.shape[0]

    mode, T, placements, e3 = _plan(path_mask)
    xT16 = np.ascontiguousarray(x.T.astype(np.float16))
    W16 = [[None] * len(Wls[l]) for l in range(N_LEVELS)]

    def wlin(l, e):
        if W16[l][e] is None:
            W16[l][e] = _linearize_w(Wls[l][e].astype(np.float16))
        return W16[l][e]

    out_full = np.zeros((B, D), dtype=np.float32)
    last_res = None

    if mode == "tail":
        Bc = 512 + T
        nc = _build((512, T), True)
        in_maps = []
        for c in range(N_CORES):
            prim, tail, w3sec_eid = placements[c]
            eids = (0, c >> 2, c >> 1, c)
            xTc = np.zeros((D, Bc), dtype=np.float16)
            xTc[:, :len(prim)] = xT16[:, prim]
            xTc[:, 512:512 + len(tail)] = xT16[:, tail]
            in_maps.append({
                "xT": xTc,
                **{f"W{l}": wlin(l, eids[l]) for l in range(N_LEVELS)},
                "W3sec": wlin(3, w3sec_eid),
                "bias": np.ascontiguousarray(
                    np.stack([bls[l][eids[l]] for l in range(N_LEVELS)])),
            })
        res = run_bass_kernel_spmd(nc, in_maps, list(range(N_CORES)),
                                   trace=_trace)
        last_res = res
        for c in range(N_CORES):
            prim, tail, _ = placements[c]
            o = res.results[c]["out"]
            out_full[prim] = o[:, :len(prim)].T.astype(np.float32)
            if len(tail):
                out_full[tail] = o[:, 512:512 + len(tail)].T.astype(np.float32)
    else:
        e3, maxg = placements
        full, rem = divmod(maxg, 512)
        chunks = (512,) * full
        if rem:
            chunks = chunks + (max(16, (rem + 7) // 8 * 8),)
        Bc = sum(chunks)
        nseg = math.ceil(maxg / Bc)
        nc = _build(chunks, False)
        core_rows = [np.nonzero(e3 == c)[0] for c in range(N_CORES)]
        wb_maps = []
        for c in range(N_CORES):
            eids = (0, c >> 2, c >> 1, c)
            wb_maps.append({
                **{f"W{l}": wlin(l, eids[l]) for l in range(N_LEVELS)},
                "bias": np.ascontiguousarray(
                    np.stack([bls[l][eids[l]] for l in range(N_LEVELS)])),
            })
        for s in range(nseg):
            in_maps = []
            for c in range(N_CORES):
                rows = core_rows[c][s * Bc:(s + 1) * Bc]
                xTc = np.zeros((D, Bc), dtype=np.float16)
                xTc[:, :len(rows)] = xT16[:, rows]
                in_maps.append({"xT": xTc, **wb_maps[c]})
            res = run_bass_kernel_spmd(nc, in_maps, list(range(N_CORES)),
                                       trace=_trace)
            last_res = res
            for c in range(N_CORES):
                rows = core_rows[c][s * Bc:(s + 1) * Bc]
                out_full[rows] = res.results[c]["out"][:, :len(rows)].T.astype(
                    np.float32)
    if _trace:
        return out_full, last_res
    return out_full
